# revision 54
# baseline (speedup 1.0000x reference)
"""Trainium2 Bass kernel for BEiT-style attention with relative position bias.

Shapes (hardcoded): x (64, 197, 768), 12 heads x 64 dim, rpb table (732, 12).

Sharding: data-parallel over batch -- 8 batches per NeuronCore, weights
replicated. Each core processes its 8 batches in 4 pairs (moving dim 394).

Per-core dataflow (all layouts chosen so no on-device transposes are needed):
  qk^T   = W_qk @ x^T        float32r matmuls, heads pair-packed on partitions
  v_nat  = x @ W_v^T         token-major V with a fused ones-column per head
  s^T    = k_h^T.T @ q_h^T   fp16, keys on partitions
  e      = exp(s^T - 5) * exp(rpb^T)    (softmax max-subtract replaced by a
                                         constant shift; exactly cancels)
  pv     = [v_h | ones].T @ e   -> rows 0:64 unnormalized out^T, row 64 colsum
  out^T  = pv[0:64] * bcast(1/colsum)
  final  = out^T.T @ W_p^T + b  float32r, token-major output
"""

import sys

if "/opt/trn_rl_repo" not in sys.path:
    sys.path.insert(0, "/opt/trn_rl_repo")

import numpy as np

import concourse.bass as bass
import concourse.mybir as mybir
import concourse.tile as tile
from concourse import bacc
from concourse.bass_utils import run_bass_kernel_spmd

F32 = mybir.dt.float32
F16 = mybir.dt.float16
F32R = mybir.dt.float32r
AF = mybir.ActivationFunctionType

B, N, C, H, HD = 64, 197, 768, 12, 64
NCORES = 8
BC = B // NCORES          # batches per core
PAIRS = BC // 2           # batch pairs per core
TP = 2 * N                # tokens per pair (394)
T = BC * N                # tokens per core (1576)
KT = C // 128             # contraction tiles (6)
SCALE = HD ** -0.5
VW = H * (HD + 1)         # v buffer width incl. ones columns (780)
EXP_SHIFT = -5.0


DEBUG = False


def _r(x):
    return x.bitcast(F32R)


def _ktile_layout(w):
    """(768, M) -> (128, 6*M) with k-tile-major columns."""
    m = w.shape[1]
    return np.ascontiguousarray(
        w.reshape(KT, 128, m).transpose(1, 0, 2).reshape(128, KT * m)
    )


def _build_program():
    nc = bacc.Bacc("TRN2", target_bir_lowering=False, debug=False,
                   num_devices=NCORES)

    xt_d = nc.declare_dram_parameter("xt", [128, PAIRS * KT * TP], F16, isOutput=False)
    wqk_d = nc.declare_dram_parameter("wqk", [128, KT * 12 * 128], F16, isOutput=False)
    wv_d = nc.declare_dram_parameter("wv", [128, KT * VW], F16, isOutput=False)
    wp_d = nc.declare_dram_parameter("wp", [128, KT * C], F16, isOutput=False)
    rpb0_d = nc.declare_dram_parameter("rpb0", [128, H * TP], F16, isOutput=False)
    qkb_d = nc.declare_dram_parameter("qkb", [128, 12], F32, isOutput=False)
    vbr_d = nc.declare_dram_parameter("vbr", [1, VW], F32, isOutput=False)
    pbr_d = nc.declare_dram_parameter("pbr", [128, 6], F32, isOutput=False)
    out_d = nc.declare_dram_parameter("out", [C, T], F32, isOutput=True)
    if DEBUG:
        dbg_es_d = nc.declare_dram_parameter("dbg_es", [128, TP], F16,
                                             isOutput=True)
        dbg_ot_d = nc.declare_dram_parameter("dbg_ot", [128, TP], F16,
                                             isOutput=True)
        dbg_qk_d = nc.declare_dram_parameter("dbg_qk", [128, TP], F16,
                                             isOutput=True)
        dbg_v_d = nc.declare_dram_parameter("dbg_v", [128, VW], F16,
                                            isOutput=True)

    from contextlib import ExitStack

    with tile.TileContext(nc) as tc, ExitStack() as ctx:
        consts = ctx.enter_context(tc.tile_pool(name="consts", bufs=1))
        xt_pool = ctx.enter_context(tc.tile_pool(name="xt", bufs=2))
        qk_pool = ctx.enter_context(tc.tile_pool(name="qk", bufs=2))
        v_pool = ctx.enter_context(tc.tile_pool(name="v", bufs=2))
        es_pool = ctx.enter_context(tc.tile_pool(name="es", bufs=1))
        ot_pool = ctx.enter_context(tc.tile_pool(name="ot", bufs=2))
        fs_pool = ctx.enter_context(tc.tile_pool(name="fs", bufs=2))
        rr_pool = ctx.enter_context(tc.tile_pool(name="rr", bufs=2))
        pvs_pool = ctx.enter_context(tc.tile_pool(name="pvs", bufs=8))
        rb_pool = ctx.enter_context(tc.tile_pool(name="rb", bufs=4))
        dram_pool = ctx.enter_context(tc.tile_pool(name="dsc", bufs=4, space="DRAM"))
        ps_mm = ctx.enter_context(tc.tile_pool(name="ps_mm", bufs=3, space="PSUM"))
        ps_sc = ctx.enter_context(tc.tile_pool(name="ps_sc", bufs=1, space="PSUM"))
        ps_pv = ctx.enter_context(tc.tile_pool(name="ps_pv", bufs=2, space="PSUM"))

        if True:
            # wqk single tile, j-major chunks so the first qk matmul group
            # only waits on a 196KB DMA; slice for (j, k) is
            # wqk_t[:, (j*KT+k)*128 : +128]
            wqk_t = consts.tile([128, 12 * KT * 128], F16, name="wqk")
            wv_t = [consts.tile([128, VW], F16, name=f"wv{k}")
                    for k in range(KT)]
            wp_t = [consts.tile([128, C], F16, name=f"wp{k}")
                    for k in range(KT)]
            nc.sync.dma_start(wqk_t[:, 0:768], wqk_d[:, 0:768])
            qkb = consts.tile([128, 12], F32)
            nc.sync.dma_start(qkb[:], qkb_d[:])
            xt0_t = [xt_pool.tile([128, TP], F16, tag=f"xt{k}",
                                  name=f"xt0{k}") for k in range(KT)]
            for k in range(KT):
                nc.sync.dma_start(xt0_t[k][:], xt_d[:, k * TP:(k + 1) * TP])
            for j in range(1, 12):
                nc.sync.dma_start(wqk_t[:, j * 768:(j + 1) * 768],
                                  wqk_d[:, j * 768:(j + 1) * 768])
            for k in range(KT):
                nc.sync.dma_start(wv_t[k][:], wv_d[:, k * VW:(k + 1) * VW])
            rpb0 = consts.tile([128, H * TP], F16)
            nc.sync.dma_start(rpb0[:], rpb0_d[:])
            for k in range(KT):
                nc.sync.dma_start(wp_t[k][:], wp_d[:, k * C:(k + 1) * C])
            vbr = consts.tile([128, VW], F32)
            _vb = vbr_d[:]
            nc.sync.dma_start(
                vbr[:],
                bass.AP(tensor=_vb.tensor, offset=_vb.offset,
                        ap=[[0, 128]] + list(_vb.ap[1:])),
            )
            pbr = consts.tile([128, 6], F32)
            nc.sync.dma_start(pbr[:], pbr_d[:])
            nb = consts.tile([128, 1], F32)
            nc.vector.memset(nb[:], EXP_SHIFT)
            # es tiles persist across half-batches; one-time zero of the
            # [69:128, N:2N] slack region (kept zero by the rpb tail rows)
            es_all = [consts.tile([128, TP], F16, name=f"es{h}")
                      for h in range(H)]
            for h in range(H):
                nc.vector.memset(es_all[h][64:128, N:2 * N], 0.0)

            def _emit_proj_j(p, ot_tiles, j, pf=None):
                if pf is None:
                    pf = ps_mm.tile([128, TP], F32, tag="mm", name="pf")
                    for k in range(KT):
                        nc.tensor.matmul(
                            pf[:],
                            wp_t[k][:, j * 128:(j + 1) * 128],
                            ot_tiles[k][:],
                            start=(k == 0), stop=(k == KT - 1),
                        )
                fs = fs_pool.tile([128, TP], F32, tag="fs", name="fs")
                nc.scalar.activation(fs[:], pf[:], AF.Identity,
                                     bias=pbr[:, j:j + 1])
                nc.sync.dma_start(
                    out_d[j * 128:(j + 1) * 128, p * TP:(p + 1) * TP],
                    fs[:],
                )

            def _make_proj(p, ot_tiles):
                def emit():
                    for j in range(6):
                        _emit_proj_j(p, ot_tiles, j)
                return emit

            # persistent scores PSUM tiles (3-deep rotation); the merged exp
            # reads the [69:128, N:2N] slack region, zeroed once here and
            # never rewritten (the rpb tail rows are zero there)
            psc_all = [ps_sc.tile([128, TP], F32, tag=f"sc{i}",
                                  name=f"sc{i}") for i in range(3)]
            for i in range(3):
                nc.vector.memset(psc_all[i][64:128, N:2 * N], 0.0)
            psc_ctr = [0]

            def emit_qk_group(xt_t, qk_tiles, j):
                pq = ps_mm.tile([128, TP], F32, tag="mm", name="pq")
                for k in range(KT):
                    nc.tensor.matmul(
                        pq[:],
                        wqk_t[:, (j * KT + k) * 128:(j * KT + k + 1) * 128],
                        xt_t[k][:],
                        start=(k == 0), stop=(k == KT - 1),
                    )
                qj = qk_pool.tile([128, TP], F16, tag=f"qk{j}", name=f"qk{j}")
                nc.vector.tensor_scalar_add(qj[:], pq[:], qkb[:, j:j + 1])
                qk_tiles.append(qj)

            def emit_v_tile(xt_t, v_tiles, b2, t):
                toff, rows = ((b2 * N, 128) if t == 0
                              else (b2 * N + 128, N - 128))
                vt = v_pool.tile([128, VW], F16, tag=f"v{b2}{t}", name=f"v{b2}{t}")
                for half in range(2):
                    hw = VW // 2
                    pv = ps_mm.tile([128, TP], F32, tag="mm", name="pv")
                    for k in range(KT):
                        nc.tensor.matmul(
                            pv[0:rows, 0:hw],
                            xt_t[k][:, toff:toff + rows],
                            wv_t[k][:, half * hw:(half + 1) * hw],
                            start=(k == 0), stop=(k == KT - 1),
                        )
                    nc.vector.tensor_add(
                        vt[0:rows, half * hw:(half + 1) * hw],
                        pv[0:rows, 0:hw],
                        vbr[0:rows, half * hw:(half + 1) * hw],
                    )
                v_tiles[(b2, t)] = vt

            state = {"norm": None}

            def build_attention(p, qk_tiles, v_tiles, ot_tiles):
                """Emission chunks for pair p's attention (both half-batches).
                qk_tiles/v_tiles are filled lazily by S1(p) thunks that run
                before these chunks; closures index them at emission time."""
                chunks = []

                def build_half(b2, boff):
                    # fresh cells per half-batch (shared-cell closures over a
                    # plain loop variable would leak b2=0 state into b2=1)
                    es_tiles = []
                    pvs_tiles = []
                    hold = {}

                    def mk_sc(h, boff):
                        def c():
                            jt, hb = h // 2, (h % 2) * 64
                            psc = psc_all[psc_ctr[0] % 3]
                            psc_ctr[0] += 1
                            kt_tile = qk_tiles[6 + jt]
                            q_rhs = qk_tiles[jt][hb:hb + 64, boff:boff + N]
                            nc.tensor.matmul(
                                psc[:, 0:N],
                                kt_tile[hb:hb + 64, boff:boff + 128],
                                q_rhs, start=True, stop=True,
                            )
                            nc.tensor.matmul(
                                psc[0:69, N:2 * N],
                                kt_tile[hb:hb + 64, boff + 128:boff + N],
                                q_rhs, start=True, stop=True,
                            )
                            es = es_all[h]
                            # single merged exp: the psc slack rows 69:128 of
                            # the tail half are zero (one-time memset), and
                            # the rpb tail rows there are zero, so the junk
                            # exp(-5) values are wiped by the mul
                            nc.scalar.activation(es[:], psc[:],
                                                 AF.Exp, bias=nb[:])
                            nc.vector.tensor_mul(
                                es[:], es[:], rpb0[:, h * TP:(h + 1) * TP])
                            if DEBUG and p == 0 and b2 == 1 and h == 0:
                                nc.sync.dma_start(dbg_es_d[:], es[:])
                            es_tiles.append(es)
                        return c

                    for h in range(H):
                        chunks.append(mk_sc(h, boff))

                    def fire_norm():
                        # deferred normalize of the previous half-batch: its
                        # DVE muls queue after this passA's rpb muls
                        if state["norm"] is not None:
                            state["norm"]()
                            state["norm"] = None
                    chunks.append(fire_norm)

                    def mk_pv(h, b2):
                        def c():
                            if h == 0:
                                hold["dsc"] = dram_pool.tile(
                                    [1, H * N], F32, tag="dsc", name="dsc")
                            es = es_tiles[h]
                            o = h % 2
                            if o == 0:
                                hold["ppv"] = ps_pv.tile([65, TP], F32,
                                                         tag="pv", name="ppv")
                            ppv = hold["ppv"]
                            coff = o * N
                            nc.tensor.matmul(
                                ppv[:, coff:coff + N],
                                v_tiles[(b2, 0)][0:128, h * 65:(h + 1) * 65],
                                es[:, 0:N], start=True, stop=False,
                            )
                            nc.tensor.matmul(
                                ppv[:, coff:coff + N],
                                v_tiles[(b2, 1)][0:69, h * 65:(h + 1) * 65],
                                es[0:69, N:2 * N],
                                start=False, stop=True,
                            )
                            if o == 1:
                                pvs = pvs_pool.tile([65, TP], F32, tag="pvs", name="pvs")
                                nc.scalar.copy(pvs[:], ppv[:])
                                pvs_tiles.append(pvs)
                                nc.gpsimd.dma_start(
                                    hold["dsc"][0:1, (h - 1) * N:(h + 1) * N],
                                    pvs[64:65, :])
                        return c

                    for h in range(H):
                        chunks.append(mk_pv(h, b2))

                    def colsum(boff=boff):
                        # DRAM hop to put 12 colsum rows on 12 partitions,
                        # one DVE reciprocal, hop back for broadcast loads
                        rsb = rr_pool.tile([H, N], F32, tag="rsb", name="rsb")
                        _d = hold["dsc"][:]
                        nc.gpsimd.dma_start(
                            rsb[:],
                            bass.AP(tensor=_d.tensor, offset=_d.offset,
                                    ap=[[N, H], [1, N]]))
                        rsr = rr_pool.tile([H, N], F32, tag="rsr", name="rsr")
                        nc.vector.reciprocal(rsr[:], rsb[:])
                        dsc2 = dram_pool.tile([H, N], F32, tag="dsc2", name="dsc2")
                        nc.gpsimd.dma_start(dsc2[:], rsr[:])
                        my_pvs = list(pvs_tiles)

                        def norm(proj_cb=None, dsc2=dsc2, my_pvs=my_pvs,
                                 boff=boff):
                            # tail norms issue rb loads on the gpsimd ring so
                            # the final out DMAs aren't queued behind them
                            dma_eng = nc.gpsimd if proj_cb is not None \
                                else nc.sync
                            for hh in range(H):
                                jt, hb = hh // 2, (hh % 2) * 64
                                rb = rb_pool.tile([64, N], F32, tag="rb",
                                                  name="rb")
                                _d2 = dsc2[:]
                                dma_eng.dma_start(
                                    rb[:],
                                    bass.AP(tensor=_d2.tensor,
                                            offset=_d2.offset + hh * N,
                                            ap=[[0, 64], [1, N]]),
                                )
                                nc.vector.tensor_mul(
                                    ot_tiles[jt][hb:hb + 64, boff:boff + N],
                                    my_pvs[hh // 2][0:64, (hh % 2) * N:
                                                    (hh % 2) * N + N],
                                    rb[:],
                                )
                                if proj_cb is not None and hh % 2 == 1:
                                    proj_cb(hh // 2)
                        state["norm"] = norm
                    chunks.append(colsum)

                for b2 in range(2):
                    build_half(b2, b2 * N)
                return chunks

            def interleave(s2, s1, _seq=[False]):
                if _seq[0]:
                    for c in s2:
                        c()
                    for f in s1:
                        f()
                    return
                n2, n1 = len(s2), len(s1)
                j = 0
                for i in range(n2):
                    s2[i]()
                    while j < n1 and j * n2 < (i + 1) * n1:
                        s1[j]()
                        j += 1
                while j < n1:
                    s1[j]()
                    j += 1

            # ---- prologue: S1(0) emitted alone
            xt_ts = {0: xt0_t}
            qk_all = {0: []}
            v_all = {0: {}}
            ot_all = {}
            for j in range(12):
                emit_qk_group(xt_ts[0], qk_all[0], j)
            for b2 in range(2):
                for t in range(2):
                    emit_v_tile(xt_ts[0], v_all[0], b2, t)
            if DEBUG:
                nc.sync.dma_start(dbg_qk_d[:], qk_all[0][0][:])
                nc.sync.dma_start(dbg_v_d[:], v_all[0][(1, 0)][:])

            for p in range(PAIRS):
                ot_all[p] = [ot_pool.tile([128, TP], F16, tag=f"ot{k}",
                                          name=f"ot{k}") for k in range(KT)]
                s2 = build_attention(p, qk_all[p], v_all[p], ot_all[p])
                s1 = []
                if p + 1 < PAIRS:
                    # issue the next pair's x^T DMAs right away
                    xt_t = [xt_pool.tile([128, TP], F16, tag=f"xt{k}",
                                         name=f"xt{k}") for k in range(KT)]
                    for k in range(KT):
                        nc.sync.dma_start(
                            xt_t[k][:],
                            xt_d[:, (p + 1) * KT * TP + k * TP:
                                 (p + 1) * KT * TP + (k + 1) * TP])
                    xt_ts[p + 1] = xt_t
                    qk_all[p + 1] = []
                    v_all[p + 1] = {}
                    for j in range(12):
                        s1.append(lambda j=j, p1=p + 1: emit_qk_group(
                            xt_ts[p1], qk_all[p1], j))
                    for b2 in range(2):
                        for t in range(2):
                            s1.append(lambda b2=b2, t=t, p1=p + 1:
                                      emit_v_tile(xt_ts[p1], v_all[p1],
                                                  b2, t))
                if p - 1 >= 0:
                    for j in range(6):
                        s1.append(lambda j=j, pm=p - 1: _emit_proj_j(
                            pm, ot_all[pm], j))
                if p == PAIRS - 1:
                    # fire norm(p-1, b1) first (its colsum chain is long
                    # done) so proj(p-1) in s1 can interleave from the start
                    s2[12]()
                    interleave(s2[:12] + s2[13:], s1)
                else:
                    interleave(s2, s1)
                if DEBUG and p == 1:
                    nc.sync.dma_start(dbg_ot_d[:], ot_all[0][0][:])

                if p == PAIRS - 1:
                    # tail: pace proj j=0 k-major behind the final norms so
                    # the PE starts the last projection as ot k-tiles finish
                    pf0 = ps_mm.tile([128, TP], F32, tag="mm", name="pf")

                    def _proj_cb(k, pf0=pf0, p=p):
                        nc.tensor.matmul(
                            pf0[:],
                            wp_t[k][:, 0:128],
                            ot_all[p][k][:],
                            start=(k == 0), stop=(k == KT - 1),
                        )

                    state["norm"](proj_cb=_proj_cb)
                    state["norm"] = None
                    _emit_proj_j(p, ot_all[p], 0, pf=pf0)
                    for j in range(1, 6):
                        _emit_proj_j(p, ot_all[p], j)

    nc.compile()
    return nc


_PROGRAM_CACHE = {}


def _get_program():
    if "nc" not in _PROGRAM_CACHE:
        _PROGRAM_CACHE["nc"] = _build_program()
    return _PROGRAM_CACHE["nc"]


def _host_prep(x, qkv_w, q_bias, v_bias, rpb_table, proj_w, proj_b,
               rel_pos_index):
    x = np.asarray(x, dtype=np.float32)
    qkv_w = np.asarray(qkv_w, dtype=np.float32)
    q_bias = np.asarray(q_bias, dtype=np.float32)
    v_bias = np.asarray(v_bias, dtype=np.float32)
    rpb_table = np.asarray(rpb_table, dtype=np.float32)
    proj_w = np.asarray(proj_w, dtype=np.float32)
    proj_b = np.asarray(proj_b, dtype=np.float32)
    rel_pos_index = np.asarray(rel_pos_index)

    w_q, w_k, w_v = qkv_w[0:C], qkv_w[C:2 * C], qkv_w[2 * C:3 * C]

    # qk^T weights: q columns pre-scaled; (768, 1536) -> j-major device:
    # dev[cr, (j*KT+k)*128+jc] = w_qkT[k*128+cr, j*128+jc]
    w_qkT = np.concatenate([w_q.T * SCALE, w_k.T], axis=1)
    wqk_dev = np.ascontiguousarray(
        w_qkT.reshape(KT, 128, 12, 128).transpose(1, 2, 0, 3)
        .reshape(128, 12 * KT * 128)).astype(np.float16)

    qkb = np.zeros((128, 12), dtype=np.float32)
    for j in range(6):
        qkb[:, j] = q_bias[j * 128:(j + 1) * 128] * SCALE

    # v weights with a zero column after each head's 64 (ones come from vbr)
    w_vT = w_v.T  # (768, 768)
    w_vT_pad = np.zeros((C, VW), dtype=np.float32)
    vbr = np.zeros((1, VW), dtype=np.float32)
    for h in range(H):
        w_vT_pad[:, h * 65:h * 65 + 64] = w_vT[:, h * 64:(h + 1) * 64]
        vbr[0, h * 65:h * 65 + 64] = v_bias[h * 64:(h + 1) * 64]
        vbr[0, h * 65 + 64] = 1.0
    wv_dev = _ktile_layout(w_vT_pad).astype(np.float16)

    wp_dev = _ktile_layout(np.ascontiguousarray(proj_w.T)).astype(np.float16)
    pbr = np.ascontiguousarray(proj_b.reshape(6, 128).T)

    # exp(rpb^T): [key, query, head]; per head [main(128,197) | tail(69,197)]
    # packed as [128, 394] with zero rows 69:128 in the tail half
    rpb_g = rpb_table[rel_pos_index.reshape(-1)].reshape(N, N, H)
    erT = np.exp(rpb_g.transpose(1, 0, 2))
    rpb0 = np.zeros((128, H * TP), dtype=np.float16)
    for h in range(H):
        rpb0[:, h * TP:h * TP + N] = erT[0:128, :, h]
        rpb0[0:69, h * TP + N:(h + 1) * TP] = erT[128:N, :, h]

    shared = {
        "wqk": wqk_dev, "wv": wv_dev, "wp": wp_dev,
        "rpb0": rpb0, "qkb": qkb, "vbr": vbr, "pbr": pbr,
    }

    in_maps = []
    for c in range(NCORES):
        xc = x[c * BC:(c + 1) * BC].reshape(T, C)
        xT = xc.T.astype(np.float16)  # (768, 1576)
        xt_dev = np.ascontiguousarray(
            xT.reshape(KT, 128, PAIRS, TP).transpose(1, 2, 0, 3)
            .reshape(128, PAIRS * KT * TP))
        in_maps.append({"xt": xt_dev, **shared})
    return in_maps


def _ensure_devices():
    import jax

    try:
        if len(jax.devices()) >= NCORES:
            return
    except Exception:
        pass
    try:
        jax.config.update("jax_platforms", "axon")
    except Exception:
        pass


def kernel(x, qkv_w, q_bias, v_bias, rpb_table, proj_w, proj_b,
           rel_pos_index, _trace=False, _trace_kwargs=None):
    _ensure_devices()
    nc = _get_program()
    in_maps = _host_prep(x, qkv_w, q_bias, v_bias, rpb_table, proj_w, proj_b,
                         rel_pos_index)
    res = run_bass_kernel_spmd(
        nc, in_maps, core_ids=list(range(NCORES)),
        trace=_trace, **(_trace_kwargs or {}),
    )
    out = np.concatenate(
        [res.results[c]["out"].reshape(C, T).T.reshape(BC, N, C)
         for c in range(NCORES)], axis=0)
    if _trace:
        kernel._last_results = res
    return out

